# revision 1
# baseline (speedup 1.0000x reference)
"""Autoregressive GRU on 8 TRN2 NeuronCores.

Data-parallel: batch B=512 is split as 64 rows per core; the small GRU
weights are replicated and the T=128 sequential loop runs locally per core.

Key algebra (Keras GRU, reset_after=True, gate order [z, r, h]):
  step 0:  inp = 0, h = x  ->  gx = b[0], gh = x @ U + b[1]
  step t>=1: inp == h      ->  gx + gh uses (W + U) for the z and r gates
so per step we need ONE matmul against a host-prefused weight matrix:
  V  = [Wr+Ur | Uh | Wh | Wz+Uz]   (steps >= 1)   [D, 4D]
  V0 = [Ur   | Uh | 0  | Uz    ]   (step 0)       [D, 4D]
with PSUM bank layout [rpre | hh | xh | zpre], then
  r = sigmoid(rpre); hhat = tanh(xh + r*hh); z = sigmoid(zpre)
  h_new = hhat + z*(h - hhat)

Perf structure (what made this fast):
- float32r matmuls: fp32 operand storage at bf16 stream rate (1 cycle/row at
  N=512), so the fused weights carry no quantization error; only the bf16
  recurrent state and gate intermediates contribute (~1e-2 rel overall).
- One PSUM tile PER GATE BANK, ordered [r, hh, xh, z]: Tile's dependency
  tracking is tile-granular, so sigmoid(r) starts right after the r bank's
  4-matmul accumulation group instead of after all 16 matmuls, and the
  r -> p -> q -> tanh chain overlaps the rest of the matmul stream.
- hT (the next step's stationary operand) is rebuilt from TWO groups of PE
  transposes, exploiting linearity of the transpose:
      hT = copy(hhat^T) ; hT += tt^T      (DVE, SBUF + PSUM operands)
  The hhat^T group runs mid-tail on the otherwise-idle PE (which also keeps
  the HAM activity monitor at K=8/8 - otherwise the per-step idle window
  re-throttles the PE to 1.2 GHz and everything runs 2x slow), and only
  tt^T + the copy/add pair remain on the recurrence-critical chain. The
  batch-major h_new add, the f32 output copy and the output DMA all run
  off-chain. (Transpose-mode matmuls do NOT accumulate in PSUM - start/stop
  flags are ignored - hence the copy+add assembly on the DVE instead.)
- A warm-up preamble of identity matmuls (no DMA dependence) flips the PE
  clock gate to K=8/8 before step 0, and two tiny regular matmuls reading
  hhat/z anchor PE activity mid-tail. (~6% of matmuls still start at the
  1.2 GHz cold clock from HAM micro-oscillation across the PSUM
  accumulation groups; no filler strategy fixes it - transpose-mode ops
  are invisible to the activity monitor, and heavy regular-matmul filler
  tips the chip into the P0 power state, 2.4 -> 2.0 GHz on everything.)
- Measured converged step = 6.5-7.1 us: 2.97 us r/hh/xh matmul stream
  (overlapping sigmoid(r) -> p) + 0.7 q + 0.8 tanh + 0.67 sigmoid(z) +
  0.49 tt + 0.35 tt^T + 0.47 hT assembly + sem hops. Pairwise collectives
  measure ~9 us each on this fabric, so cross-core gate splitting with a
  per-step h exchange can never pay.
"""

import numpy as np
import ml_dtypes

B, D, T = 512, 512, 128
NCORES = 8
BLOC = B // NCORES  # 64
P = 128
KC = D // P  # 4 K-chunks
GW = 4 * D  # 2048 gate columns: [r | hh | xh | z]

_BF16 = ml_dtypes.bfloat16

# set by test harness to capture a profile; harmless when False
TRACE = False
TMPDIR = None
LAST = {}


def _prepare_weights(W, U, b):
    """Host-side fusion. Returns (V, V0, bias) in math layout."""
    Wz, Wr, Wh = W[:, :D], W[:, D : 2 * D], W[:, 2 * D :]
    Uz, Ur, Uh = U[:, :D], U[:, D : 2 * D], U[:, 2 * D :]
    V = np.concatenate([Wr + Ur, Uh, Wh, Wz + Uz], axis=1)  # [D, GW]
    V0 = np.concatenate([Ur, Uh, np.zeros_like(Wh), Uz], axis=1)
    b0, b1 = b[0], b[1]
    bias = np.concatenate(
        [b0[D : 2 * D] + b1[D : 2 * D], b1[2 * D :], b0[2 * D :], b0[:D] + b1[:D]]
    )  # [GW], order [r | hh | xh | z]
    return V, V0, bias


def _dev_layout(V):
    # V_dev[p, k*GW + j] = V[k*128 + p, j]
    return np.ascontiguousarray(
        V.reshape(KC, P, GW).transpose(1, 0, 2).reshape(P, KC * GW)
    )


_CACHE = {}


def _build(has_bias: bool):
    import concourse.mybir as mybir
    import concourse.tile as tile
    from concourse import bacc
    from concourse.masks import make_identity

    f32 = mybir.dt.float32
    f32r = mybir.dt.float32r
    bf16 = mybir.dt.bfloat16
    AF = mybir.ActivationFunctionType

    nc = bacc.Bacc(
        "TRN2", target_bir_lowering=False, debug=False, num_devices=NCORES
    )
    v0_d = nc.dram_tensor("v0", [P, KC * GW], f32r, kind="ExternalInput").ap()
    v_d = nc.dram_tensor("v", [P, KC * GW], f32r, kind="ExternalInput").ap()
    h0_d = nc.dram_tensor("h0", [BLOC, D], bf16, kind="ExternalInput").ap()
    h0T_d = nc.dram_tensor("h0T", [P, KC * BLOC], f32r, kind="ExternalInput").ap()
    if has_bias:
        bias_d = nc.dram_tensor("bias", [BLOC, GW], f32, kind="ExternalInput").ap()
    out_d = nc.dram_tensor("out", [BLOC, T, D], f32, kind="ExternalOutput").ap()

    with tile.TileContext(nc) as tc:
        with (
            tc.tile_pool(name="const", bufs=1) as cpool,
            tc.tile_pool(name="state", bufs=2) as spool,
            tc.tile_pool(name="work", bufs=3) as wpool,
            tc.tile_pool(name="outp", bufs=3) as opool,
            tc.tile_pool(name="gates", bufs=1, space="PSUM") as gpool,
            tc.tile_pool(name="trp", bufs=2, space="PSUM") as trpool,
            tc.tile_pool(name="warm", bufs=1, space="PSUM") as warmpool,
            tc.tile_pool(name="scr", bufs=1, space="PSUM") as scrpool,
        ):
            v0_sb = cpool.tile([P, KC * GW], f32r, tag="v0")
            v_sb = cpool.tile([P, KC * GW], f32r, tag="v")
            ident = cpool.tile([BLOC, BLOC], bf16, tag="ident")
            nc.sync.dma_start(v0_sb[:], v0_d[:])
            make_identity(nc, ident[:])

            h = spool.tile([BLOC, D], bf16, tag="h")
            hT = spool.tile([P, KC * BLOC], f32r, tag="hT")
            nc.sync.dma_start(h[:], h0_d[:])
            nc.sync.dma_start(hT[:], h0T_d[:])
            nc.sync.dma_start(v_sb[:], v_d[:])
            if has_bias:
                bias_sb = cpool.tile([BLOC, GW], f32, tag="bias")
                nc.sync.dma_start(bias_sb[:], bias_d[:])

            # PE warm-up: dense transpose work that depends only on the
            # locally-built identity (not on any DMA) flips the HAM clock
            # gate to K=8/8 while the weight DMAs are still in flight.
            wu = trpool.tile([P, KC * BLOC], bf16, tag="trp", name="wu")
            for i in range(24):
                nc.tensor.matmul(
                    wu[:BLOC, (i % KC) * BLOC : (i % KC + 1) * BLOC],
                    ident[:],
                    ident[:],
                    is_transpose=True,
                    start=True,
                    stop=True,
                )

            for t in range(T):
                vsb = v0_sb if t == 0 else v_sb
                last = t == T - 1
                # one PSUM tile per gate bank: [r | hh | xh | z]
                gb = [
                    gpool.tile([BLOC, 512], f32, tag=f"g{n}", name=f"g{n}")
                    for n in range(4)
                ]
                def bank_mms(n, stop=True):
                    for k in range(KC):
                        nc.tensor.matmul(
                            gb[n][:],
                            hT[:, k * BLOC : (k + 1) * BLOC],
                            vsb[:, k * GW + n * 512 : k * GW + (n + 1) * 512],
                            start=(k == 0),
                            stop=(k == KC - 1) and stop,
                        )
                    if has_bias:
                        nc.vector.tensor_add(
                            gb[n][:], gb[n][:], bias_sb[:, n * 512 : (n + 1) * 512]
                        )

                bank_mms(0)  # rpre
                r = wpool.tile([BLOC, D], bf16, tag="r", name="r")
                nc.scalar.activation(r[:], gb[0][:], AF.Sigmoid)
                bank_mms(1)  # hh
                p = wpool.tile([BLOC, D], bf16, tag="p", name="p")
                nc.vector.tensor_mul(p[:], r[:], gb[1][:])
                bank_mms(2)  # xh
                # q goes into the retired r-gate PSUM bank (free after
                # sigmoid(r)/p consumed it): ScalarE reads PSUM faster than
                # SBUF, so tanh starts ~50-150 ns sooner
                q = gb[0]
                nc.vector.tensor_add(q[:], p[:], gb[2][:])
                bank_mms(3)  # zpre
                if not last:
                    scr = scrpool.tile([P, KC * BLOC], bf16, tag="scr", name="scr")
                    for k in range(3):
                        nc.tensor.matmul(
                            scr[:, k * BLOC : (k + 1) * BLOC],
                            p[:, k * P : (k + 1) * P],
                            ident[:],
                            is_transpose=True,
                            start=True,
                            stop=True,
                        )
                hhat = wpool.tile([BLOC, D], bf16, tag="hhat", name="hhat")
                nc.scalar.activation(hhat[:], q[:], AF.Tanh)

                if not last:
                    # trpA = hhat^T: real mid-tail PE activity (keeps the HAM
                    # clock gate warm) that feeds the hT rebuild below
                    trpA = warmpool.tile(
                        [P, KC * BLOC], bf16, tag="warm", name="trpA"
                    )
                    for k in range(KC):
                        nc.tensor.matmul(
                            trpA[:, k * BLOC : (k + 1) * BLOC],
                            hhat[:, k * P : (k + 1) * P],
                            ident[:],
                            is_transpose=True,
                            start=True,
                            stop=True,
                        )

                s = wpool.tile([BLOC, D], bf16, tag="s", name="s")
                nc.vector.tensor_sub(s[:], h[:], hhat[:])
                if not last:
                    for k in range(2):
                        nc.tensor.matmul(
                            scr[:, k * BLOC : (k + 1) * BLOC],
                            s[:, k * P : (k + 1) * P],
                            ident[:],
                            is_transpose=True,
                            start=True,
                            stop=True,
                        )
                z = wpool.tile([BLOC, D], bf16, tag="z", name="z")
                nc.scalar.activation(z[:], gb[3][:], AF.Sigmoid)
                if not last:
                    for k in range(2):
                        nc.tensor.matmul(
                            scr[:, (2 + k) * BLOC : (3 + k) * BLOC],
                            z[:, k * P : (k + 1) * P],
                            ident[:],
                            is_transpose=True,
                            start=True,
                            stop=True,
                        )
                tt = wpool.tile([BLOC, D], bf16, tag="t", name="tt")
                nc.vector.tensor_mul(tt[:], z[:], s[:])

                if not last:
                    # trpB = tt^T; then hT_new = trpA^ + trpB^ = h_new^T
                    # (transpose is linear), so the h_new add, the f32 output
                    # copy and the DMA all run OFF the recurrence chain
                    trpB = trpool.tile([P, KC * BLOC], bf16, tag="trp", name="trpB")
                    for k in range(KC):
                        nc.tensor.matmul(
                            trpB[:, k * BLOC : (k + 1) * BLOC],
                            tt[:, k * P : (k + 1) * P],
                            ident[:],
                            is_transpose=True,
                            start=True,
                            stop=True,
                        )
                    hT_new = spool.tile([P, KC * BLOC], f32r, tag="hT")
                    nc.vector.tensor_copy(hT_new[:], trpA[:])
                    nc.vector.tensor_add(hT_new[:], hT_new[:], trpB[:])
                    hT = hT_new

                h_new = spool.tile([BLOC, D], bf16, tag="h")
                nc.vector.tensor_add(h_new[:], hhat[:], tt[:])
                of = opool.tile([BLOC, D], f32, tag="of", name="of")
                nc.scalar.copy(of[:], h_new[:])
                nc.sync.dma_start(out_d[:, t, :], of[:])
                h = h_new

    nc.compile()
    return nc


def kernel(x, W, U, b):
    from concourse.bass_utils import run_bass_kernel_spmd

    x = np.asarray(x, dtype=np.float32)
    W = np.asarray(W, dtype=np.float32)
    U = np.asarray(U, dtype=np.float32)
    b = np.asarray(b, dtype=np.float32)

    V, V0, bias = _prepare_weights(W, U, b)
    has_bias = bool(np.any(bias != 0.0))
    v_dev = _dev_layout(V).astype(np.float32)
    v0_dev = _dev_layout(V0).astype(np.float32)

    key = ("gru", has_bias)
    if key not in _CACHE:
        _CACHE[key] = _build(has_bias)
    nc = _CACHE[key]

    in_maps = []
    for i in range(NCORES):
        xs = x[i * BLOC : (i + 1) * BLOC]  # [64, 512]
        m = {
            "v0": v0_dev,
            "v": v_dev,
            "h0": xs.astype(_BF16),
            "h0T": np.ascontiguousarray(
                xs.astype(_BF16)
                .astype(np.float32)
                .reshape(BLOC, KC, P)
                .transpose(2, 1, 0)
                .reshape(P, KC * BLOC)
            ),
        }
        if has_bias:
            m["bias"] = np.ascontiguousarray(
                np.broadcast_to(bias[None, :], (BLOC, GW))
            ).astype(np.float32)
        in_maps.append(m)

    res = run_bass_kernel_spmd(
        nc, in_maps, core_ids=list(range(NCORES)), trace=TRACE, tmpdir=TMPDIR
    )
    LAST["exec_time_ns"] = res.exec_time_ns
    LAST["results"] = res
    out = np.concatenate([res.results[i]["out"] for i in range(NCORES)], axis=0)
    return out.astype(np.float32)



# revision 8
# speedup vs baseline: 1.0410x; 1.0410x over previous
"""Autoregressive GRU on 8 TRN2 NeuronCores.

Data-parallel: batch B=512 is split as 64 rows per core; the small GRU
weights are replicated and the T=128 sequential loop runs locally per core.

Key algebra (Keras GRU, reset_after=True, gate order [z, r, h]):
  step 0:  inp = 0, h = x  ->  gx = b[0], gh = x @ U + b[1]
  step t>=1: inp == h      ->  gx + gh uses (W + U) for the z and r gates
so per step we need ONE matmul against a host-prefused weight matrix
  V  = [Wr+Ur | Wz+Uz | Uh | Wh]   (steps >= 1)   [D, 4D]
  V0 = [Ur   | Uz    | Uh | 0 ]   (step 0)       [D, 4D]
with per-gate PSUM banks in order [r, z, hh, xh], then
  r = sigmoid(rpre); z = sigmoid(zpre); hhat = tanh(xh + r*hh)
  h_new = (1-z)*hhat + z*h

Perf structure (v2 - col-tiled, fold-128 layout):
- Each M=64 matmul only fills half the 128-col PE array.  We issue the two
  256-wide halves of every gate row-block as a tile_position=(0,0)/(0,64)
  pair: the pair runs CONCURRENTLY on the two column halves of the array
  (4ns stagger), so a gate bank costs 4x~107ns instead of 4x~215ns.
- The pair's outputs land on PSUM partitions 0:64 and 64:128, i.e. every
  gate tensor is [128, 256] ("folded": partition = fold*64 + batch,
  col = feature % 256).  All elementwise work therefore runs at FD=256 on
  128 partitions - half the instruction time of the baseline's [64, 512].
- Bank order [r, z, hh, xh]: both sigmoids, u = z*h (GPSIMD) and w = 1-z
  run under the hh/xh matmul stream; the post-stream chain is only
  q = p+xh -> tanh -> m = w*hhat -> h_new = m+u -> 4 PE transposes ->
  one CAST to the fp16 stationary hT.
- Moving operands stay fp16 (exact weights, 1 cyc/row at N=256); the
  recurrent state is fp16 (~1e-2 rel overall).
- Warm-up identity transposes + two tiny regular matmuls mid-tail keep the
  PE HAM activity monitor from re-throttling the clock to 1.2 GHz.
"""

import numpy as np
import ml_dtypes

B, D, T = 512, 512, 128
NCORES = 8
BLOC = B // NCORES  # 64
P = 128
KC = D // P  # 4 K-chunks
FH = 256  # fold width (free dim of every folded [128, 256] tensor)
GW = 4 * D  # 2048 gate columns: [r | z | hh | xh]

_FP16 = np.float16

# set by test harness to capture a profile; harmless when False
TRACE = False
TMPDIR = None
LAST = {}
# ablation flags (for debugging; all True in production)
ANCHORS = True
WARMUP = True
GPSIMD_U = True
DO_TRP = True


def _prepare_weights(W, U, b):
    """Host-side fusion. Gate order [r | z | hh | xh]."""
    Wz, Wr, Wh = W[:, :D], W[:, D : 2 * D], W[:, 2 * D :]
    Uz, Ur, Uh = U[:, :D], U[:, D : 2 * D], U[:, 2 * D :]
    V = np.concatenate([Wr + Ur, Wz + Uz, Uh, Wh], axis=1)  # [D, GW]
    V0 = np.concatenate([Ur, Uz, Uh, np.zeros_like(Wh)], axis=1)
    b0, b1 = b[0], b[1]
    bias = np.concatenate(
        [b0[D : 2 * D] + b1[D : 2 * D], b0[:D] + b1[:D], b1[2 * D :], b0[2 * D :]]
    )  # [GW], order [r | z | hh | xh]
    return V, V0, bias


def _dev_layout(V):
    # V_dev[p, ((k*4+g)*2+hf)*FH + c] = V[k*128+p, g*512 + hf*256 + c]
    return np.ascontiguousarray(
        V.reshape(KC, P, 4, 2, FH).transpose(1, 0, 2, 3, 4).reshape(P, KC * GW)
    )


def _fold_bias(bias):
    # folded per-gate [P, FH]: row p = fold*64+b (same for all b), col c
    out = np.zeros((4, P, FH), dtype=np.float32)
    for g in range(4):
        for hf in range(2):
            blk = bias[g * 512 + hf * 256 : g * 512 + (hf + 1) * 256]
            out[g, hf * BLOC : (hf + 1) * BLOC, :] = blk[None, :]
    return out


_CACHE = {}


def _build(has_bias: bool, T=T):
    import concourse.mybir as mybir
    import concourse.tile as tile
    from concourse import bacc
    from concourse.masks import make_identity

    f32 = mybir.dt.float32
    fp16 = mybir.dt.float16
    AF = mybir.ActivationFunctionType
    ALU = mybir.AluOpType

    nc = bacc.Bacc(
        "TRN2", target_bir_lowering=False, debug=False, num_devices=NCORES
    )
    v0_d = nc.dram_tensor("v0", [P, KC * GW], fp16, kind="ExternalInput").ap()
    v_d = nc.dram_tensor("v", [P, KC * GW], fp16, kind="ExternalInput").ap()
    h0_d = nc.dram_tensor("h0", [P, FH], fp16, kind="ExternalInput").ap()
    h0T_d = nc.dram_tensor("h0T", [P, KC * BLOC], fp16, kind="ExternalInput").ap()
    if has_bias:
        bias_d = nc.dram_tensor("bias", [4, P, FH], f32, kind="ExternalInput").ap()
    out_d = nc.dram_tensor("out", [P, T, FH], f32, kind="ExternalOutput").ap()  # noqa: T param

    with tile.TileContext(nc) as tc:
        with (
            tc.tile_pool(name="const", bufs=1) as cpool,
            tc.tile_pool(name="state", bufs=2) as spool,
            tc.tile_pool(name="work", bufs=2) as wpool,
            tc.tile_pool(name="outp", bufs=3) as opool,
            tc.tile_pool(name="gates", bufs=1, space="PSUM") as gpool,
            tc.tile_pool(name="trp", bufs=1, space="PSUM") as trpool,
            tc.tile_pool(name="warm", bufs=1, space="PSUM") as warmpool,
            tc.tile_pool(name="anc", bufs=1, space="PSUM") as ancpool,
        ):
            v0_sb = cpool.tile([P, KC * GW], fp16, tag="v0")
            v_sb = cpool.tile([P, KC * GW], fp16, tag="v")
            ident = cpool.tile([P, BLOC], fp16, tag="ident")
            nc.sync.dma_start(v0_sb[:], v0_d[:])
            make_identity(nc, ident[:BLOC, :])
            make_identity(nc, ident[BLOC:, :])

            h = spool.tile([P, FH], fp16, tag="h")
            hTs = [
                spool.tile([P, BLOC], fp16, tag=f"hT{k}", name=f"hT{k}")
                for k in range(KC)
            ]
            nc.sync.dma_start(h[:], h0_d[:])
            for k in range(KC):
                nc.sync.dma_start(hTs[k][:], h0T_d[:, k * BLOC : (k + 1) * BLOC])
            nc.sync.dma_start(v_sb[:], v_d[:])
            if has_bias:
                bias_sb = cpool.tile([4, P, FH], f32, tag="bias")
                nc.sync.dma_start(bias_sb[:], bias_d[:])

            # PE warm-up: dense transpose work that depends only on the
            # locally-built identity (not on any DMA) flips the HAM clock
            # gate to K=8/8 while the weight DMAs are still in flight.
            wu = warmpool.tile([P, KC * BLOC], fp16, tag="warm", name="wu")
            for i in range(24 if WARMUP else 0):
                nc.tensor.matmul(
                    wu[:BLOC, (i % KC) * BLOC : (i % KC + 1) * BLOC],
                    ident[:BLOC, :],
                    ident[:BLOC, :],
                    is_transpose=True,
                    start=True,
                    stop=True,
                )

            for t in range(T):
                vsb = v0_sb if t == 0 else v_sb
                last = t == T - 1
                # one folded PSUM tile per gate bank: [r, z, hh, xh]
                gb = [
                    gpool.tile([P, FH], f32, tag=f"g{n}", name=f"g{n}")
                    for n in range(4)
                ]

                def bank_mms(g):
                    for k in range(KC):
                        for hf in range(2):
                            nc.tensor.matmul(
                                gb[g][hf * BLOC : (hf + 1) * BLOC, :],
                                hTs[k][:],
                                vsb[
                                    :,
                                    ((k * 4 + g) * 2 + hf) * FH : ((k * 4 + g) * 2 + hf + 1) * FH,
                                ],
                                start=(k == 0),
                                stop=(k == KC - 1),
                                skip_group_check=True,
                            )
                    if has_bias:
                        nc.vector.tensor_add(gb[g][:], gb[g][:], bias_sb[g])

                bank_mms(0)  # rpre
                r = wpool.tile([P, FH], fp16, tag="r", name="r")
                nc.scalar.activation(r[:], gb[0][:], AF.Sigmoid)
                bank_mms(1)  # zpre
                zt = wpool.tile([P, FH], fp16, tag="z", name="zt")
                nc.scalar.activation(zt[:], gb[1][:], AF.Sigmoid)
                # u = z*h and w = 1-z run under the hh/xh matmul stream
                u = wpool.tile([P, FH], fp16, tag="u", name="u")
                (nc.gpsimd if GPSIMD_U else nc.vector).tensor_mul(u[:], zt[:], h[:])
                bank_mms(2)  # hh
                p = wpool.tile([P, FH], fp16, tag="p", name="p")
                nc.vector.tensor_mul(p[:], r[:], gb[2][:])
                bank_mms(3)  # xh
                # q goes into the retired r-gate PSUM bank (ScalarE reads
                # PSUM faster than SBUF, so tanh starts sooner)
                q = gb[0]
                nc.vector.tensor_add(q[:], p[:], gb[3][:])
                w = wpool.tile([P, FH], fp16, tag="w", name="w")
                nc.vector.tensor_scalar(w[:], zt[:], -1.0, 1.0, ALU.mult, ALU.add)
                hhat = wpool.tile([P, FH], fp16, tag="hhat", name="hhat")
                nc.scalar.activation(hhat[:], q[:], AF.Tanh)

                if not last and ANCHORS:
                    # tiny regular matmuls mid-tail anchor PE activity so the
                    # HAM clock gate stays at K=8/8 across the tail window
                    anc = ancpool.tile([BLOC, BLOC], f32, tag="anc", name="anc")
                    nc.tensor.matmul(
                        anc[:],
                        hhat[:BLOC, :BLOC],
                        hhat[:BLOC, :BLOC],
                        start=True,
                        stop=True,
                        skip_group_check=True,
                    )

                m = wpool.tile([P, FH], fp16, tag="m", name="m")
                nc.vector.tensor_mul(m[:], w[:], hhat[:])
                h_new = spool.tile([P, FH], fp16, tag="h")
                nc.vector.tensor_add(h_new[:], m[:], u[:])

                if not last:
                    if ANCHORS:
                        anc2 = ancpool.tile([BLOC, BLOC], f32, tag="anc", name="anc2")
                        nc.tensor.matmul(
                            anc2[:],
                            m[:BLOC, :BLOC],
                            m[:BLOC, :BLOC],
                            start=True,
                            stop=True,
                            skip_group_check=True,
                        )
                    # hT_new = h_new^T via 4 PE transposes + per-chunk CASTs
                    # into 4 separate stationary tiles (a single whole-tile
                    # fp16 PSUM copy spanning the 4 transpose groups faults
                    # the NEFF at runtime; per-chunk copies also give the
                    # next step's k-MMs finer-grained dependencies)
                    trp = trpool.tile([P, KC * BLOC], fp16, tag="trp", name="trp")
                    hTs_new = []
                    for k in range(KC):
                        fold = k // 2
                        coff = (k % 2) * P
                        nc.tensor.matmul(
                            trp[:, k * BLOC : (k + 1) * BLOC],
                            h_new[fold * BLOC : (fold + 1) * BLOC, coff : coff + P],
                            ident[fold * BLOC : (fold + 1) * BLOC, :],
                            is_transpose=True,
                            start=True,
                            stop=True,
                        )
                        hTk = spool.tile([P, BLOC], fp16, tag=f"hT{k}")
                        nc.vector.tensor_copy(hTk[:], trp[:, k * BLOC : (k + 1) * BLOC])
                        hTs_new.append(hTk)
                    if DO_TRP:
                        hTs = hTs_new

                of = opool.tile([P, FH], f32, tag="of", name="of")
                nc.scalar.copy(of[:], h_new[:])
                nc.sync.dma_start(out_d[:, t, :], of[:])
                h = h_new

    nc.compile()
    return nc


def kernel(x, W, U, b):
    from concourse.bass_utils import run_bass_kernel_spmd

    x = np.asarray(x, dtype=np.float32)
    W = np.asarray(W, dtype=np.float32)
    U = np.asarray(U, dtype=np.float32)
    b = np.asarray(b, dtype=np.float32)

    V, V0, bias = _prepare_weights(W, U, b)
    has_bias = bool(np.any(bias != 0.0))
    v_dev = _dev_layout(V).astype(_FP16)
    v0_dev = _dev_layout(V0).astype(_FP16)

    key = ("gru_v3_fp16", has_bias, T, ANCHORS, WARMUP, GPSIMD_U, DO_TRP)
    if key not in _CACHE:
        _CACHE[key] = _build(has_bias, T)
    nc = _CACHE[key]

    in_maps = []
    for i in range(NCORES):
        xs = x[i * BLOC : (i + 1) * BLOC]  # [64, 512]
        xb = xs.astype(_FP16)
        xf = xb
        m = {
            "v0": v0_dev,
            "v": v_dev,
            # folded batch-major state: [p = fold*64+b, c] = x[b, fold*256+c]
            "h0": np.ascontiguousarray(
                xb.reshape(BLOC, 2, FH).transpose(1, 0, 2).reshape(P, FH)
            ),
            # transposed state: [p, k*64+b] = x[b, k*128+p]
            "h0T": np.ascontiguousarray(
                xf.reshape(BLOC, KC, P).transpose(2, 1, 0).reshape(P, KC * BLOC)
            ),
        }
        if has_bias:
            m["bias"] = _fold_bias(bias)
        in_maps.append(m)

    res = run_bass_kernel_spmd(
        nc, in_maps, core_ids=list(range(NCORES)), trace=TRACE, tmpdir=TMPDIR
    )
    LAST["exec_time_ns"] = res.exec_time_ns
    LAST["results"] = res
    outs = []
    for i in range(NCORES):
        o = res.results[i]["out"]  # [P, T, FH]
        outs.append(
            o.reshape(2, BLOC, T, FH).transpose(1, 2, 0, 3).reshape(BLOC, T, D)
        )
    out = np.concatenate(outs, axis=0)
    return out.astype(np.float32)


# revision 11
# speedup vs baseline: 1.1848x; 1.1381x over previous
"""Autoregressive GRU on 8 TRN2 NeuronCores.

Data-parallel: batch B=512 is split as 64 rows per core; the small GRU
weights are replicated and the T=128 sequential loop runs locally per core.

Key algebra (Keras GRU, reset_after=True, gate order [z, r, h]):
  step 0:  inp = 0, h = x  ->  gx = b[0], gh = x @ U + b[1]
  step t>=1: inp == h      ->  gx + gh uses (W + U) for the z and r gates
so per step we need ONE matmul against a host-prefused weight matrix
  V  = [Wr+Ur | Wz+Uz | Uh | Wh]   (steps >= 1)   [D, 4D]
  V0 = [Ur   | Uz    | Uh | 0 ]   (step 0)       [D, 4D]
with per-gate PSUM banks in order [r, z, hh, xh], then
  r = sigmoid(rpre); z = sigmoid(zpre); hhat = tanh(xh + r*hh)
  h_new = (1-z)*hhat + z*h

Perf structure (v2 - col-tiled, fold-128 layout):
- Each M=64 matmul only fills half the 128-col PE array.  We issue the two
  256-wide halves of every gate row-block as a tile_position=(0,0)/(0,64)
  pair: the pair runs CONCURRENTLY on the two column halves of the array
  (4ns stagger), so a gate bank costs 4x~107ns instead of 4x~215ns.
- The pair's outputs land on PSUM partitions 0:64 and 64:128, i.e. every
  gate tensor is [128, 256] ("folded": partition = fold*64 + batch,
  col = feature % 256).  All elementwise work therefore runs at FD=256 on
  128 partitions - half the instruction time of the baseline's [64, 512].
- Bank order [r, z, hh, xh]: both sigmoids, u = z*h (GPSIMD) and w = 1-z
  run under the hh/xh matmul stream; the post-stream chain is only
  q = p+xh -> tanh -> m = w*hhat -> h_new = m+u -> 4 PE transposes ->
  one CAST to the fp16 stationary hT.
- Moving operands stay fp16 (exact weights, 1 cyc/row at N=256); the
  recurrent state is fp16 (~1e-2 rel overall).
- Warm-up identity transposes + two tiny regular matmuls mid-tail keep the
  PE HAM activity monitor from re-throttling the clock to 1.2 GHz.
"""

import numpy as np
import ml_dtypes

B, D, T = 512, 512, 128
NCORES = 8
BLOC = B // NCORES  # 64
P = 128
KC = D // P  # 4 K-chunks
FH = 256  # fold width (free dim of every folded [128, 256] tensor)
GW = 4 * D  # 2048 gate columns: [r | z | hh | xh]

_FP16 = np.float16

# set by test harness to capture a profile; harmless when False
TRACE = False
TMPDIR = None
LAST = {}
# ablation flags (for debugging; all True in production)
ANCHORS = True
NDUMMY = 12
WARMUP = True
GPSIMD_U = True
DO_TRP = True


def _prepare_weights(W, U, b):
    """Host-side fusion. Gate order [r | z | hh | xh]."""
    Wz, Wr, Wh = W[:, :D], W[:, D : 2 * D], W[:, 2 * D :]
    Uz, Ur, Uh = U[:, :D], U[:, D : 2 * D], U[:, 2 * D :]
    V = np.concatenate([Wr + Ur, Wz + Uz, Uh, Wh], axis=1)  # [D, GW]
    V0 = np.concatenate([Ur, Uz, Uh, np.zeros_like(Wh)], axis=1)
    b0, b1 = b[0], b[1]
    bias = np.concatenate(
        [b0[D : 2 * D] + b1[D : 2 * D], b0[:D] + b1[:D], b1[2 * D :], b0[2 * D :]]
    )  # [GW], order [r | z | hh | xh]
    return V, V0, bias


def _dev_layout(V):
    # V_dev[p, ((k*4+g)*2+hf)*FH + c] = V[k*128+p, g*512 + hf*256 + c]
    return np.ascontiguousarray(
        V.reshape(KC, P, 4, 2, FH).transpose(1, 0, 2, 3, 4).reshape(P, KC * GW)
    )


def _fold_bias(bias):
    # folded per-gate [P, FH]: row p = fold*64+b (same for all b), col c
    out = np.zeros((4, P, FH), dtype=np.float32)
    for g in range(4):
        for hf in range(2):
            blk = bias[g * 512 + hf * 256 : g * 512 + (hf + 1) * 256]
            out[g, hf * BLOC : (hf + 1) * BLOC, :] = blk[None, :]
    return out


_CACHE = {}


def _build(has_bias: bool, T=T):
    import concourse.mybir as mybir
    import concourse.tile as tile
    from concourse import bacc
    from concourse.masks import make_identity

    f32 = mybir.dt.float32
    fp16 = mybir.dt.float16
    AF = mybir.ActivationFunctionType
    ALU = mybir.AluOpType

    nc = bacc.Bacc(
        "TRN2", target_bir_lowering=False, debug=False, num_devices=NCORES
    )
    v0_d = nc.dram_tensor("v0", [P, KC * GW], fp16, kind="ExternalInput").ap()
    v_d = nc.dram_tensor("v", [P, KC * GW], fp16, kind="ExternalInput").ap()
    h0_d = nc.dram_tensor("h0", [P, FH], fp16, kind="ExternalInput").ap()
    h0T_d = nc.dram_tensor("h0T", [P, KC * BLOC], fp16, kind="ExternalInput").ap()
    if has_bias:
        bias_d = nc.dram_tensor("bias", [4, P, FH], f32, kind="ExternalInput").ap()
    out_d = nc.dram_tensor("out", [P, T, FH], f32, kind="ExternalOutput").ap()  # noqa: T param

    with tile.TileContext(nc) as tc:
        with (
            tc.tile_pool(name="const", bufs=1) as cpool,
            tc.tile_pool(name="state", bufs=2) as spool,
            tc.tile_pool(name="work", bufs=2) as wpool,
            tc.tile_pool(name="outp", bufs=3) as opool,
            tc.tile_pool(name="gates", bufs=1, space="PSUM") as gpool,
            tc.tile_pool(name="trp", bufs=1, space="PSUM") as trpool,
            tc.tile_pool(name="warm", bufs=1, space="PSUM") as warmpool,
            tc.tile_pool(name="anc", bufs=1, space="PSUM") as ancpool,
        ):
            v0_sb = cpool.tile([P, KC * GW], fp16, tag="v0")
            v_sb = cpool.tile([P, KC * GW], fp16, tag="v")
            ident = cpool.tile([P, BLOC], fp16, tag="ident")
            nc.sync.dma_start(v0_sb[:], v0_d[:])
            make_identity(nc, ident[:BLOC, :])
            make_identity(nc, ident[BLOC:, :])

            h = spool.tile([P, FH], fp16, tag="h")
            hTs = [
                spool.tile([P, BLOC], fp16, tag=f"hT{k}", name=f"hT{k}")
                for k in range(KC)
            ]
            nc.sync.dma_start(h[:], h0_d[:])
            for k in range(KC):
                nc.sync.dma_start(hTs[k][:], h0T_d[:, k * BLOC : (k + 1) * BLOC])
            nc.sync.dma_start(v_sb[:], v_d[:])
            if has_bias:
                bias_sb = cpool.tile([4, P, FH], f32, tag="bias")
                nc.sync.dma_start(bias_sb[:], bias_d[:])

            # PE warm-up: dense transpose work that depends only on the
            # locally-built identity (not on any DMA) flips the HAM clock
            # gate to K=8/8 while the weight DMAs are still in flight.
            wu = warmpool.tile([P, KC * BLOC], fp16, tag="warm", name="wu")
            for i in range(24 if WARMUP else 0):
                nc.tensor.matmul(
                    wu[:BLOC, (i % KC) * BLOC : (i % KC + 1) * BLOC],
                    ident[:BLOC, :],
                    ident[:BLOC, :],
                    is_transpose=True,
                    start=True,
                    stop=True,
                )

            for t in range(T):
                vsb = v0_sb if t == 0 else v_sb
                last = t == T - 1
                # one folded PSUM tile per gate bank: [r, z, hh, xh]
                gb = [
                    gpool.tile([P, FH], f32, tag=f"g{n}", name=f"g{n}")
                    for n in range(4)
                ]

                KORD = (0, 2, 1, 3)  # chunk order matching the cast order

                def bank_mms(g):
                    for ki, k in enumerate(KORD):
                        for hf in range(2):
                            nc.tensor.matmul(
                                gb[g][hf * BLOC : (hf + 1) * BLOC, :],
                                hTs[k][:],
                                vsb[
                                    :,
                                    ((k * 4 + g) * 2 + hf) * FH : ((k * 4 + g) * 2 + hf + 1) * FH,
                                ],
                                start=(ki == 0),
                                stop=(ki == KC - 1),
                                skip_group_check=True,
                            )
                    if has_bias:
                        nc.vector.tensor_add(gb[g][:], gb[g][:], bias_sb[g])

                bank_mms(0)  # rpre
                r = wpool.tile([P, FH], fp16, tag="r", name="r")
                nc.scalar.activation(r[:], gb[0][:], AF.Sigmoid)
                bank_mms(1)  # zpre
                zt = wpool.tile([P, FH], fp16, tag="z", name="zt")
                nc.scalar.activation(zt[:], gb[1][:], AF.Sigmoid)
                # u = z*h and w = 1-z run under the hh/xh matmul stream
                u = wpool.tile([P, FH], fp16, tag="u", name="u")
                (nc.gpsimd if GPSIMD_U else nc.vector).tensor_mul(u[:], zt[:], h[:])
                bank_mms(2)  # hh
                p = wpool.tile([P, FH], fp16, tag="p", name="p")
                nc.vector.tensor_mul(p[:], r[:], gb[2][:])
                bank_mms(3)  # xh
                # q goes into the retired r-gate PSUM bank (ScalarE reads
                # PSUM faster than SBUF, so tanh starts sooner)
                q = gb[0]
                nc.vector.tensor_add(q[:], p[:], gb[3][:])
                w = wpool.tile([P, FH], fp16, tag="w", name="w")
                nc.vector.tensor_scalar(w[:], zt[:], -1.0, 1.0, ALU.mult, ALU.add)
                hhat = wpool.tile([P, FH], fp16, tag="hhat", name="hhat")
                nc.scalar.activation(hhat[:], q[:], AF.Tanh)

                if not last and ANCHORS:
                    # Dummy matmul pairs fill the PE pipe across the
                    # q->tanh->m->h_new tail window.  Without them the PE
                    # idles ~2us every step and the HAM activity monitor
                    # parks the clock at K=4/8 (1.2 GHz) for the whole
                    # kernel - every matmul then runs at half speed.  The
                    # dummies re-read the z-gate slices against the current
                    # stationaries into a scratch PSUM tile that is never
                    # read.
                    dmy = ancpool.tile([P, FH], f32, tag="anc", name="dmy")
                    for i in range(NDUMMY):
                        k = KORD[i % KC]
                        for hf in range(2):
                            nc.tensor.matmul(
                                dmy[hf * BLOC : (hf + 1) * BLOC, :],
                                hTs[k][:],
                                vsb[
                                    :,
                                    ((k * 4 + 1) * 2 + hf) * FH : ((k * 4 + 1) * 2 + hf + 1) * FH,
                                ],
                                start=True,
                                stop=True,
                                skip_group_check=True,
                            )

                m = wpool.tile([P, FH], fp16, tag="m", name="m")
                nc.vector.tensor_mul(m[:], w[:], hhat[:])
                h_new = spool.tile([P, FH], fp16, tag="h")
                nc.vector.tensor_add(h_new[:], m[:], u[:])

                if not last:
                    # hT_new = h_new^T via 4 PE transposes + per-chunk CASTs
                    # into 4 separate stationary tiles (a single whole-tile
                    # fp16 PSUM copy spanning the 4 transpose groups faults
                    # the NEFF at runtime; per-chunk copies also give the
                    # next step's k-MMs finer-grained dependencies)
                    trp = trpool.tile([P, KC * BLOC], fp16, tag="trp", name="trp")
                    hTs_new = [None] * KC
                    for k in KORD:
                        fold = k // 2
                        coff = (k % 2) * P
                        nc.tensor.matmul(
                            trp[:, k * BLOC : (k + 1) * BLOC],
                            h_new[fold * BLOC : (fold + 1) * BLOC, coff : coff + P],
                            ident[fold * BLOC : (fold + 1) * BLOC, :],
                            is_transpose=True,
                            start=True,
                            stop=True,
                        )
                        hTk = spool.tile([P, BLOC], fp16, tag=f"hT{k}")
                        nc.vector.tensor_copy(hTk[:], trp[:, k * BLOC : (k + 1) * BLOC])
                        hTs_new[k] = hTk
                    if DO_TRP:
                        hTs = hTs_new

                of = opool.tile([P, FH], f32, tag="of", name="of")
                nc.scalar.copy(of[:], h_new[:])
                nc.sync.dma_start(out_d[:, t, :], of[:])
                h = h_new

    nc.compile()
    return nc


def kernel(x, W, U, b):
    from concourse.bass_utils import run_bass_kernel_spmd

    x = np.asarray(x, dtype=np.float32)
    W = np.asarray(W, dtype=np.float32)
    U = np.asarray(U, dtype=np.float32)
    b = np.asarray(b, dtype=np.float32)

    V, V0, bias = _prepare_weights(W, U, b)
    has_bias = bool(np.any(bias != 0.0))
    v_dev = _dev_layout(V).astype(_FP16)
    v0_dev = _dev_layout(V0).astype(_FP16)

    key = ("gru_v4_fp16", has_bias, T, ANCHORS, NDUMMY, WARMUP, GPSIMD_U, DO_TRP)
    if key not in _CACHE:
        _CACHE[key] = _build(has_bias, T)
    nc = _CACHE[key]

    in_maps = []
    for i in range(NCORES):
        xs = x[i * BLOC : (i + 1) * BLOC]  # [64, 512]
        xb = xs.astype(_FP16)
        xf = xb
        m = {
            "v0": v0_dev,
            "v": v_dev,
            # folded batch-major state: [p = fold*64+b, c] = x[b, fold*256+c]
            "h0": np.ascontiguousarray(
                xb.reshape(BLOC, 2, FH).transpose(1, 0, 2).reshape(P, FH)
            ),
            # transposed state: [p, k*64+b] = x[b, k*128+p]
            "h0T": np.ascontiguousarray(
                xf.reshape(BLOC, KC, P).transpose(2, 1, 0).reshape(P, KC * BLOC)
            ),
        }
        if has_bias:
            m["bias"] = _fold_bias(bias)
        in_maps.append(m)

    res = run_bass_kernel_spmd(
        nc, in_maps, core_ids=list(range(NCORES)), trace=TRACE, tmpdir=TMPDIR
    )
    LAST["exec_time_ns"] = res.exec_time_ns
    LAST["results"] = res
    outs = []
    for i in range(NCORES):
        o = res.results[i]["out"]  # [P, T, FH]
        outs.append(
            o.reshape(2, BLOC, T, FH).transpose(1, 2, 0, 3).reshape(BLOC, T, D)
        )
    out = np.concatenate(outs, axis=0)
    return out.astype(np.float32)
